# revision 3
# baseline (speedup 1.0000x reference)
"""Trainium2 Bass kernel for nn_MultiHeadAttentionLayer (edge-wise MHA with
global softmax over the edge dimension).

Strategy (8 NeuronCores, data-parallel over edges):
  - Host shards the E=250000 edges into 8 shards of 31250, zero-padded to
    31744 (62 chunks x 512 edges), and pre-transposes each shard so the
    feature dimension lands on SBUF partitions ([128, E] / [32, E]).
  - Pass A (per chunk): QT = wq.T@xiT, KET = wk.T@xjT + we.T@eaT (+bias),
    VT = wv.T@xjT (+bias, kept resident in SBUF), P = (QT+bq)*KET,
    S8 = Hsum.T@P (per-head dot products), exp(S8/4) -> DRAM scratch with
    per-chunk partial Z sums (ACT accumulate).
  - One AllReduce(add) of Z[8] across the 8 cores.  Since scores are O(1),
    softmax max-subtraction is unnecessary; 1/Z is folded into wo's rows.
  - Pass B (per chunk): E128 = Hrep.T@exp (replicate per-head weights to all
    16 lanes of the head), U = E128 * VT, outT = wo'.T@U + bo -> DRAM.
  - Host gathers and transposes back to [E, 128].

All matmuls use the PE's single-pass fp32r mode (full rate vs 4x-slower
fp32); end-to-end relative error ~2e-4.
"""
import os
import sys

for _p in ("/opt/trn_rl_repo", "/root/.axon_site/_ro/trn_rl_repo"):
    if os.path.isdir(_p) and _p not in sys.path:
        sys.path.append(_p)

import numpy as np
import concourse.bacc as bacc
import concourse.tile as tile
import concourse.mybir as mybir
from concourse.bass_utils import run_bass_kernel_spmd

F32 = mybir.dt.float32
F32R = mybir.dt.float32r
AF = mybir.ActivationFunctionType
ALU = mybir.AluOpType

E_FULL = 250000
NCORES = 8
ES = E_FULL // NCORES          # 31250 edges per core
CH = 512                       # chunk size (PSUM bank width for fp32)
NCH = (ES + CH - 1) // CH      # 62 chunks
EP = NCH * CH                  # 31744 padded edges per core
TAIL = ES - (NCH - 1) * CH     # 338 valid edges in the last chunk
D = 128
NH = 8
DK = 16

_CACHE = {}


def _build():
    if "nc" in _CACHE:
        return _CACHE["nc"]

    nc = bacc.Bacc(num_devices=NCORES)

    t_xiT = nc.dram_tensor("xiT", [D, EP], F32R, kind="ExternalInput")
    t_xjT = nc.dram_tensor("xjT", [D, EP], F32R, kind="ExternalInput")
    t_eaT = nc.dram_tensor("eaT", [32, EP], F32R, kind="ExternalInput")
    t_pk = nc.dram_tensor("pk", [D, 784], F32R, kind="ExternalInput")
    t_out = nc.dram_tensor("outT", [D, EP], F32, kind="ExternalOutput")

    with tile.TileContext(nc) as tc:
        with (
            tc.tile_pool(name="per", bufs=1) as per,      # persistent
            tc.tile_pool(name="wk", bufs=3) as wk,        # streaming loads
            tc.tile_pool(name="mid", bufs=2) as mid,      # intermediates
            tc.tile_pool(name="dram", bufs=1, space="DRAM") as dram,
        ):
            s_pk = per.tile([D, 784], F32R)
            nc.sync.dma_start(s_pk[:], t_pk[:])
            s_wq = s_pk[:, 0:128]
            s_wk = s_pk[:, 128:256]
            s_wv = s_pk[:, 256:384]
            s_wo = s_pk[:, 384:512]
            s_we = s_pk[0:32, 512:640]
            s_hsum = s_pk[:, 640:648]
            s_bq = s_pk[:, 648:649].bitcast(F32)
            s_bkbe = s_pk[:, 649:650].bitcast(F32)
            s_bv = s_pk[:, 650:651].bitcast(F32)
            s_bo = s_pk[:, 651:652].bitcast(F32)
            s_hrep = s_pk[0:8, 656:784]

            v_full = per.tile([D, EP], F32)      # resident V^T
            zparts = per.tile([NH, NCH], F32)    # per-chunk Z partials
            d_exp = dram.tile([NH, EP], F32R)    # exp scores scratch

            # ---------------- pass A ----------------
            psA_ctx = tc.tile_pool(name="psA", bufs=1, space="PSUM")
            psA = psA_ctx.__enter__()
            for c in range(NCH):
                sl = slice(c * CH, (c + 1) * CH)
                s_xi = wk.tile([D, CH], F32R, tag="xi")
                nc.sync.dma_start(s_xi[:], t_xiT[:, sl])
                s_xj = wk.tile([D, CH], F32R, tag="xj")
                nc.sync.dma_start(s_xj[:], t_xjT[:, sl])
                s_ea = wk.tile([32, CH], F32R, tag="ea")
                nc.sync.dma_start(s_ea[:], t_eaT[:, sl])

                p_q = psA.tile([D, CH], F32, tag="pq", bufs=2)
                nc.tensor.matmul(p_q[:], s_wq, s_xi[:], start=True, stop=True)
                p_ke = psA.tile([D, CH], F32, tag="pke", bufs=2)
                nc.tensor.matmul(p_ke[:], s_wk, s_xj[:], start=True, stop=False)
                nc.tensor.matmul(p_ke[:], s_we, s_ea[:], start=False, stop=True)
                p_v = psA.tile([D, CH], F32, tag="pv", bufs=2)
                nc.tensor.matmul(p_v[:], s_wv, s_xj[:], start=True, stop=True)

                s_ke = mid.tile([D, CH], F32, tag="ke")
                nc.scalar.activation(s_ke[:], p_ke[:], AF.Identity,
                                     bias=s_bkbe, scale=1.0)
                s_p = mid.tile([D, CH], F32R, tag="p")
                nc.vector.scalar_tensor_tensor(s_p[:], p_q[:], s_bq, s_ke[:],
                                               op0=ALU.add, op1=ALU.mult)
                # V^T chunk -> resident SBUF (bias folded in)
                nc.vector.tensor_scalar(v_full[:, sl], p_v[:], s_bv, None,
                                        op0=ALU.add)

                p_s8 = psA.tile([NH, CH], F32, tag="ps8", bufs=2)
                nc.tensor.matmul(p_s8[:], s_hsum, s_p[:], start=True, stop=True)
                s_exp = mid.tile([NH, CH], F32R, tag="exp")
                if c < NCH - 1:
                    nc.scalar.activation(s_exp[:], p_s8[:], AF.Exp, bias=0.0,
                                         scale=0.25,
                                         accum_out=zparts[:, c:c + 1])
                else:
                    # padded tail: exp, zero the pad, then reduce for Z
                    nc.scalar.activation(s_exp[:], p_s8[:], AF.Exp, bias=0.0,
                                         scale=0.25)
                    nc.vector.memset(s_exp[:, TAIL:CH].bitcast(F32), 0.0)
                    nc.vector.tensor_reduce(zparts[:, c:c + 1],
                                            s_exp[:].bitcast(F32),
                                            axis=mybir.AxisListType.X,
                                            op=ALU.add)
                nc.sync.dma_start(d_exp[:, sl], s_exp[:])

            psA_ctx.__exit__(None, None, None)
            psB_ctx = tc.tile_pool(name="psB", bufs=1, space="PSUM")
            psB = psB_ctx.__enter__()

            # ---------------- global Z ----------------
            s_zl = per.tile([NH, 1], F32)
            nc.vector.tensor_reduce(s_zl[:], zparts[:],
                                    axis=mybir.AxisListType.X, op=ALU.add)
            d_zin = dram.tile([NH, 1], F32)
            d_zout = dram.tile([NH, 1], F32)
            nc.gpsimd.dma_start(d_zin[:], s_zl[:])
            nc.gpsimd.collective_compute(
                "AllReduce", ALU.add,
                replica_groups=[list(range(NCORES))],
                ins=[d_zin.opt()],
                outs=[d_zout.opt()],
            )
            s_z = per.tile([NH, 1], F32)
            nc.gpsimd.dma_start(s_z[:], d_zout[:])
            s_rz = per.tile([NH, 1], F32)
            nc.vector.reciprocal(s_rz[:], s_z[:])
            p_ch = psB.tile([D, 1], F32, tag="pe", bufs=2)
            nc.tensor.matmul(p_ch[:], s_hrep.bitcast(F32), s_rz[:],
                             start=True, stop=True)
            s_chd = per.tile([D, 1], F32)
            nc.scalar.activation(s_chd[:], p_ch[:], AF.Copy)
            s_wo2 = per.tile([D, D], F32R)
            nc.vector.tensor_scalar(s_wo2[:], s_wo, s_chd[:], None,
                                    op0=ALU.mult)

            # ---------------- pass B ----------------
            for c in range(NCH):
                sl = slice(c * CH, (c + 1) * CH)
                s_eb = wk.tile([NH, CH], F32R, tag="eb")
                nc.sync.dma_start(s_eb[:], d_exp[:, sl])
                p_e = psB.tile([D, CH], F32, tag="pe", bufs=2)
                nc.tensor.matmul(p_e[:], s_hrep, s_eb[:], start=True, stop=True)
                s_u = mid.tile([D, CH], F32R, tag="u")
                nc.vector.tensor_tensor(s_u[:], p_e[:], v_full[:, sl],
                                        op=ALU.mult)
                p_o = psB.tile([D, CH], F32, tag="pout", bufs=2)
                nc.tensor.matmul(p_o[:], s_wo2[:], s_u[:], start=True, stop=True)
                s_o = mid.tile([D, CH], F32, tag="o")
                nc.scalar.activation(s_o[:], p_o[:], AF.Identity, bias=s_bo,
                                     scale=1.0)
                nc.sync.dma_start(t_out[:, sl], s_o[:])
            psB_ctx.__exit__(None, None, None)

    nc.compile()
    _CACHE["nc"] = nc
    return nc


def _pack_constants(wq, bq, wk, bk, wv, bv, we, be, wo, bo):
    Hsum = np.zeros((D, NH), np.float32)
    for hd in range(D):
        Hsum[hd, hd // DK] = 1.0
    pk = np.zeros((D, 784), np.float32)
    pk[:, 0:128] = wq
    pk[:, 128:256] = wk
    pk[:, 256:384] = wv
    pk[:, 384:512] = wo
    pk[:32, 512:640] = we
    pk[:, 640:648] = Hsum
    pk[:, 648] = bq
    pk[:, 649] = bk + be
    pk[:, 650] = bv
    pk[:, 651] = bo
    pk[:8, 656:784] = Hsum.T
    return pk


def _run(inputs, trace=False):
    x_i = np.asarray(inputs["x_i"], np.float32)
    x_j = np.asarray(inputs["x_j"], np.float32)
    ea = np.asarray(inputs["edge_attr"], np.float32)
    pk = _pack_constants(
        np.asarray(inputs["wq"], np.float32), np.asarray(inputs["bq"], np.float32),
        np.asarray(inputs["wk"], np.float32), np.asarray(inputs["bk"], np.float32),
        np.asarray(inputs["wv"], np.float32), np.asarray(inputs["bv"], np.float32),
        np.asarray(inputs["we"], np.float32), np.asarray(inputs["be"], np.float32),
        np.asarray(inputs["wo"], np.float32), np.asarray(inputs["bo"], np.float32),
    )

    in_maps = []
    for c in range(NCORES):
        sl = slice(c * ES, (c + 1) * ES)
        xiT = np.zeros((D, EP), np.float32)
        xiT[:, :ES] = x_i[sl].T
        xjT = np.zeros((D, EP), np.float32)
        xjT[:, :ES] = x_j[sl].T
        eaT = np.zeros((32, EP), np.float32)
        eaT[:, :ES] = ea[sl].T
        in_maps.append(dict(xiT=xiT, xjT=xjT, eaT=eaT, pk=pk))

    nc = _build()
    res = run_bass_kernel_spmd(nc, in_maps, list(range(NCORES)), trace=trace)

    out = np.empty((E_FULL, D), np.float32)
    for c in range(NCORES):
        sl = slice(c * ES, (c + 1) * ES)
        out[sl] = res.results[c]["outT"][:, :ES].T
    return out, res.exec_time_ns


def kernel(**inputs) -> np.ndarray:
    return _run(inputs)[0]


# revision 5
# speedup vs baseline: 1.3255x; 1.3255x over previous
"""Trainium2 Bass kernel for nn_MultiHeadAttentionLayer (edge-wise MHA with
global softmax over the edge dimension).

Strategy (8 NeuronCores, data-parallel over edges):
  - Host shards the E=250000 edges into 8 shards of 31250, zero-padded to
    31744 (62 chunks x 512 edges), pre-transposes each shard so the feature
    dimension lands on SBUF partitions ([128, E] / [32, E]) and casts the
    activations + weights to bf16 (the PE's full-rate dtype; fp32 matmul is
    4x slower and fp32 weight loads defeat fast-weight-load).
  - Pass A (per 512-edge chunk): QT = wq.T@xiT, KET = wk.T@xjT + we.T@eaT
    (+bias via ACT), VT = wv.T@xjT (+bias, kept resident in SBUF bf16),
    P = (QT+bq)*KET on DVE, S8 = Hsum.T@P (per-head dot products),
    exp(S8/4) -> resident SBUF bf16 with per-chunk partial Z sums (ACT
    accumulate).
  - One tiny AllReduce(add) of Z[8] across the 8 cores.  Scores are O(1)
    (inputs are N(0,1), weights uniform(+-1/sqrt(d))), so softmax
    max-subtraction is unnecessary; 1/Z is folded into wo's rows.
  - Pass B (per chunk): E128 = Hrep.T@exp (replicate per-head weights to
    the 16 lanes of each head), U = E128 * VT, outT = wo'.T@U + bo -> DRAM
    in fp32.
  - Host gathers and transposes back to [E, 128].
"""
import os
import sys

for _p in ("/opt/trn_rl_repo", "/root/.axon_site/_ro/trn_rl_repo"):
    if os.path.isdir(_p) and _p not in sys.path:
        sys.path.append(_p)

import numpy as np
import ml_dtypes
import concourse.bacc as bacc
import concourse.tile as tile
import concourse.mybir as mybir
from concourse.bass_utils import run_bass_kernel_spmd

F32 = mybir.dt.float32
BF16 = mybir.dt.bfloat16
AF = mybir.ActivationFunctionType
ALU = mybir.AluOpType
BF = ml_dtypes.bfloat16

E_FULL = 250000
NCORES = 8
ES = E_FULL // NCORES          # 31250 edges per core
CH = 512                       # chunk size (PSUM bank width)
NCH = (ES + CH - 1) // CH      # 62 chunks
EP = NCH * CH                  # 31744 padded edges per core
TAIL = ES - (NCH - 1) * CH     # 338 valid edges in the last chunk
D = 128
NH = 8
DK = 16
XW = 1024                      # xi/xj DMA batch width (2 chunks)
EW = 2048                      # ea DMA batch width (4 chunks)

_CACHE = {}


def _build():
    if "nc" in _CACHE:
        return _CACHE["nc"]

    nc = bacc.Bacc(num_devices=NCORES)

    t_xiT = nc.dram_tensor("xiT", [D, EP], BF16, kind="ExternalInput")
    t_xjT = nc.dram_tensor("xjT", [D, EP], BF16, kind="ExternalInput")
    t_eaT = nc.dram_tensor("eaT", [32, EP], BF16, kind="ExternalInput")
    t_pkb = nc.dram_tensor("pkb", [D, 784], BF16, kind="ExternalInput")
    t_pkf = nc.dram_tensor("pkf", [D, 136], F32, kind="ExternalInput")
    t_out = nc.dram_tensor("outT", [D, EP], F32, kind="ExternalOutput")

    with tile.TileContext(nc) as tc:
        with (
            tc.tile_pool(name="per", bufs=1) as per,      # persistent
            tc.tile_pool(name="wk", bufs=2) as wk,        # streaming loads
            tc.tile_pool(name="mid", bufs=2) as mid,      # intermediates
            tc.tile_pool(name="dram", bufs=1, space="DRAM") as dram,
        ):
            s_pkb = per.tile([D, 784], BF16)
            nc.sync.dma_start(s_pkb[:], t_pkb[:])
            s_wq = s_pkb[:, 0:128]
            s_wk = s_pkb[:, 128:256]
            s_wv = s_pkb[:, 256:384]
            s_wo = s_pkb[:, 384:512]
            s_we = s_pkb[0:32, 512:640]
            s_hsum = s_pkb[:, 640:648]
            s_hrep = s_pkb[0:8, 656:784]

            s_pkf = per.tile([D, 136], F32)
            nc.sync.dma_start(s_pkf[:], t_pkf[:])
            s_bq = s_pkf[:, 0:1]
            s_bkbe = s_pkf[:, 1:2]
            s_bv = s_pkf[:, 2:3]
            s_bo = s_pkf[:, 3:4]
            s_hrepf = s_pkf[0:8, 8:136]

            v_full = per.tile([D, EP], BF16)     # resident V^T
            e_full = per.tile([NH, EP], BF16)    # resident exp scores
            zparts = per.tile([NH, NCH], F32)    # per-chunk Z partials

            # ---------------- pass A ----------------
            psA_ctx = tc.tile_pool(name="psA", bufs=1, space="PSUM")
            psA = psA_ctx.__enter__()
            for c in range(NCH):
                sl = slice(c * CH, (c + 1) * CH)
                if c % (XW // CH) == 0:
                    s_xi = wk.tile([D, XW], BF16, tag="xi")
                    nc.sync.dma_start(s_xi[:], t_xiT[:, c * CH:c * CH + XW])
                    s_xj = wk.tile([D, XW], BF16, tag="xj")
                    nc.sync.dma_start(s_xj[:], t_xjT[:, c * CH:c * CH + XW])
                if c % (EW // CH) == 0:
                    ew = min(EW, EP - c * CH)
                    s_ea = wk.tile([32, EW], BF16, tag="ea")
                    nc.sync.dma_start(s_ea[:, :ew], t_eaT[:, c * CH:c * CH + ew])
                xsl = slice((c % (XW // CH)) * CH, (c % (XW // CH)) * CH + CH)
                esl = slice((c % (EW // CH)) * CH, (c % (EW // CH)) * CH + CH)

                p_q = psA.tile([D, CH], F32, tag="pq", bufs=2)
                nc.tensor.matmul(p_q[:], s_wq, s_xi[:, xsl], start=True, stop=True)
                p_ke = psA.tile([D, CH], F32, tag="pke", bufs=2)
                nc.tensor.matmul(p_ke[:], s_wk, s_xj[:, xsl], start=True, stop=False)
                nc.tensor.matmul(p_ke[:], s_we, s_ea[:, esl], start=False, stop=True)
                p_v = psA.tile([D, CH], F32, tag="pv", bufs=2)
                nc.tensor.matmul(p_v[:], s_wv, s_xj[:, xsl], start=True, stop=True)

                s_ke = mid.tile([D, CH], BF16, tag="ke")
                nc.scalar.activation(s_ke[:], p_ke[:], AF.Identity,
                                     bias=s_bkbe, scale=1.0)
                s_p = mid.tile([D, CH], BF16, tag="p")
                nc.vector.scalar_tensor_tensor(s_p[:], p_q[:], s_bq, s_ke[:],
                                               op0=ALU.add, op1=ALU.mult)
                # V^T chunk -> resident SBUF (bias folded in)
                nc.vector.tensor_scalar(v_full[:, sl], p_v[:], s_bv, None,
                                        op0=ALU.add)

                p_s8 = psA.tile([NH, CH], F32, tag="ps8", bufs=2)
                nc.tensor.matmul(p_s8[:], s_hsum, s_p[:], start=True, stop=True)
                if c < NCH - 1:
                    nc.scalar.activation(e_full[:, sl], p_s8[:], AF.Exp,
                                         bias=0.0, scale=0.25,
                                         accum_out=zparts[:, c:c + 1])
                else:
                    # padded tail: exp, zero the pad, then reduce for Z
                    nc.scalar.activation(e_full[:, sl], p_s8[:], AF.Exp,
                                         bias=0.0, scale=0.25)
                    nc.vector.memset(e_full[:, c * CH + TAIL:(c + 1) * CH], 0.0)
                    nc.vector.tensor_reduce(zparts[:, c:c + 1], e_full[:, sl],
                                            axis=mybir.AxisListType.X,
                                            op=ALU.add)

            psA_ctx.__exit__(None, None, None)
            psB_ctx = tc.tile_pool(name="psB", bufs=1, space="PSUM")
            psB = psB_ctx.__enter__()

            # ---------------- global Z ----------------
            s_zl = per.tile([NH, 1], F32)
            nc.vector.tensor_reduce(s_zl[:], zparts[:],
                                    axis=mybir.AxisListType.X, op=ALU.add)
            d_zin = dram.tile([NH, 1], F32)
            d_zout = dram.tile([NH, 1], F32)
            nc.gpsimd.dma_start(d_zin[:], s_zl[:])
            nc.gpsimd.collective_compute(
                "AllReduce", ALU.add,
                replica_groups=[list(range(NCORES))],
                ins=[d_zin.opt()],
                outs=[d_zout.opt()],
            )
            s_z = per.tile([NH, 1], F32)
            nc.gpsimd.dma_start(s_z[:], d_zout[:])
            s_rz = per.tile([NH, 1], F32)
            nc.vector.reciprocal(s_rz[:], s_z[:])
            p_ch = psB.tile([D, 1], F32, tag="pe", bufs=2)
            nc.tensor.matmul(p_ch[:], s_hrepf, s_rz[:], start=True, stop=True)
            s_chd = per.tile([D, 1], F32)
            nc.scalar.activation(s_chd[:], p_ch[:], AF.Copy)
            s_wo2 = per.tile([D, D], BF16)
            nc.vector.tensor_scalar(s_wo2[:], s_wo, s_chd[:], None,
                                    op0=ALU.mult)

            # ---------------- pass B ----------------
            for c in range(NCH):
                sl = slice(c * CH, (c + 1) * CH)
                p_e = psB.tile([D, CH], F32, tag="pe", bufs=2)
                nc.tensor.matmul(p_e[:], s_hrep, e_full[:, sl],
                                 start=True, stop=True)
                s_u = mid.tile([D, CH], BF16, tag="u")
                nc.vector.tensor_tensor(s_u[:], p_e[:], v_full[:, sl],
                                        op=ALU.mult)
                p_o = psB.tile([D, CH], F32, tag="pout", bufs=2)
                nc.tensor.matmul(p_o[:], s_wo2[:], s_u[:], start=True, stop=True)
                s_o = mid.tile([D, CH], F32, tag="o")
                nc.scalar.activation(s_o[:], p_o[:], AF.Identity, bias=s_bo,
                                     scale=1.0)
                nc.sync.dma_start(t_out[:, sl], s_o[:])
            psB_ctx.__exit__(None, None, None)

    nc.compile()
    _CACHE["nc"] = nc
    return nc


def _pack_constants(wq, bq, wk, bk, wv, bv, we, be, wo, bo):
    Hsum = np.zeros((D, NH), np.float32)
    for hd in range(D):
        Hsum[hd, hd // DK] = 1.0
    pkb = np.zeros((D, 784), np.float32)
    pkb[:, 0:128] = wq
    pkb[:, 128:256] = wk
    pkb[:, 256:384] = wv
    pkb[:, 384:512] = wo
    pkb[:32, 512:640] = we
    pkb[:, 640:648] = Hsum
    pkb[:8, 656:784] = Hsum.T
    pkf = np.zeros((D, 136), np.float32)
    pkf[:, 0] = bq
    pkf[:, 1] = bk + be
    pkf[:, 2] = bv
    pkf[:, 3] = bo
    pkf[:8, 8:136] = Hsum.T
    return pkb.astype(BF), pkf


def _run(inputs, trace=False):
    x_i = np.asarray(inputs["x_i"], np.float32)
    x_j = np.asarray(inputs["x_j"], np.float32)
    ea = np.asarray(inputs["edge_attr"], np.float32)
    pkb, pkf = _pack_constants(
        np.asarray(inputs["wq"], np.float32), np.asarray(inputs["bq"], np.float32),
        np.asarray(inputs["wk"], np.float32), np.asarray(inputs["bk"], np.float32),
        np.asarray(inputs["wv"], np.float32), np.asarray(inputs["bv"], np.float32),
        np.asarray(inputs["we"], np.float32), np.asarray(inputs["be"], np.float32),
        np.asarray(inputs["wo"], np.float32), np.asarray(inputs["bo"], np.float32),
    )

    in_maps = []
    for c in range(NCORES):
        sl = slice(c * ES, (c + 1) * ES)
        xiT = np.zeros((D, EP), BF)
        xiT[:, :ES] = x_i[sl].T.astype(BF)
        xjT = np.zeros((D, EP), BF)
        xjT[:, :ES] = x_j[sl].T.astype(BF)
        eaT = np.zeros((32, EP), BF)
        eaT[:, :ES] = ea[sl].T.astype(BF)
        in_maps.append(dict(xiT=xiT, xjT=xjT, eaT=eaT, pkb=pkb, pkf=pkf))

    nc = _build()
    res = run_bass_kernel_spmd(nc, in_maps, list(range(NCORES)), trace=trace)

    out = np.empty((E_FULL, D), np.float32)
    for c in range(NCORES):
        sl = slice(c * ES, (c + 1) * ES)
        out[sl] = res.results[c]["outT"][:, :ES].T
    return out, res.exec_time_ns


def kernel(**inputs) -> np.ndarray:
    return _run(inputs)[0]


# revision 7
# speedup vs baseline: 1.4238x; 1.0742x over previous
"""Trainium2 Bass kernel for nn_MultiHeadAttentionLayer (edge-wise MHA with
global softmax over the edge dimension).

Strategy (8 NeuronCores, data-parallel over edges):
  - Host shards E=250000 edges into 8 shards of 31250, zero-padded to 31744
    (62 chunks x 512), pre-transposed so features land on SBUF partitions,
    and cast to bf16 (PE full-rate dtype + fast weight load).
  - The KE bias (bk+be) is folded into the edge-attr matmul: host appends a
    ones-row to edge_attr^T ([33, E]) and a bias row to we ([33, 128]).
  - Pass A (per 512-edge chunk): QT = wq.T@xiT, KET = wk.T@xjT+weA.T@eaA,
    VT = wv.T@xjT (+bv via ACT, resident SBUF bf16), KE copy to SBUF (DVE),
    P = (QT+bq)*KET (DVE), S8 = Hsum.T@P.  The S8 matmul for chunk c is
    issued in iteration c+1 so the in-order PE never waits on the DVE/ACT
    chain (keeps the PE dense and HAM-warm).  exp(S8/4) runs once per chunk
    PAIR on a [8,1024] PSUM tile -> resident SBUF bf16 + partial Z sums.
  - One AllReduce(add) of Z[8].  Scores are O(1) so no max-subtraction;
    1/Z is folded into wo's rows (wo2, computed off the PE critical path:
    Z -> replicated [128,1] via a broadcast DMA -> DVE reciprocal -> DVE
    scale).  The first PREB chunks of pass B (E128 = Hrep.T@exp, U = E128*V)
    are issued before any wo2-dependent matmul so they overlap the
    collective.
  - Pass B (per chunk): U = E128 * VT (DVE), outT = wo2.T@U + bo -> DRAM
    fp32; the out matmul for chunk c is deferred one iteration like S8.
  - Host gathers and transposes back to [E, 128].
"""
import os
import sys

for _p in ("/opt/trn_rl_repo", "/root/.axon_site/_ro/trn_rl_repo"):
    if os.path.isdir(_p) and _p not in sys.path:
        sys.path.append(_p)

import numpy as np
import ml_dtypes
import concourse.bacc as bacc
import concourse.tile as tile
import concourse.mybir as mybir
from concourse.bass_utils import run_bass_kernel_spmd

F32 = mybir.dt.float32
BF16 = mybir.dt.bfloat16
AF = mybir.ActivationFunctionType
ALU = mybir.AluOpType
BF = ml_dtypes.bfloat16

E_FULL = 250000
NCORES = 8
ES = E_FULL // NCORES          # 31250 edges per core
CH = 512                       # chunk size (PSUM bank width)
NCH = (ES + CH - 1) // CH      # 62 chunks
EP = NCH * CH                  # 31744 padded edges per core
D = 128
NH = 8
DK = 16
XW = 1024                      # xi/xj DMA batch width (2 chunks)
EW = 2048                      # ea DMA batch width (4 chunks)
NPAIR = NCH // 2               # 31 exp pairs
PTAIL = ES - (NPAIR - 1) * 2 * CH   # valid edges in last pair (530)
PREB = 16                      # pass-B chunks prefetched across collective

_CACHE = {}


def _build():
    if "nc" in _CACHE:
        return _CACHE["nc"]

    nc = bacc.Bacc(num_devices=NCORES)

    t_xiT = nc.dram_tensor("xiT", [D, EP], BF16, kind="ExternalInput")
    t_xjT = nc.dram_tensor("xjT", [D, EP], BF16, kind="ExternalInput")
    t_eaT = nc.dram_tensor("eaT", [33, EP], BF16, kind="ExternalInput")
    t_pkb = nc.dram_tensor("pkb", [D, 784], BF16, kind="ExternalInput")
    t_pkf = nc.dram_tensor("pkf", [D, 8], F32, kind="ExternalInput")
    t_out = nc.dram_tensor("outT", [D, EP], F32, kind="ExternalOutput")

    with tile.TileContext(nc) as tc:
        with (
            tc.tile_pool(name="per", bufs=1) as per,      # persistent
            tc.tile_pool(name="wk", bufs=2) as wk,        # streaming loads
            tc.tile_pool(name="mid", bufs=2) as mid,      # intermediates
            tc.tile_pool(name="dram", bufs=1, space="DRAM") as dram,
        ):
            s_pkb = per.tile([D, 784], BF16)
            nc.sync.dma_start(s_pkb[:], t_pkb[:])
            s_wq = s_pkb[:, 0:128]
            s_wk = s_pkb[:, 128:256]
            s_wv = s_pkb[:, 256:384]
            s_wo = s_pkb[:, 384:512]
            s_wea = s_pkb[0:33, 512:640]     # [we ; bk+be]
            s_hsum = s_pkb[:, 640:648]
            s_hrep = s_pkb[0:8, 656:784]

            s_pkf = per.tile([D, 8], F32)
            nc.sync.dma_start(s_pkf[:], t_pkf[:])
            s_bq = s_pkf[:, 0:1]
            s_bv = s_pkf[:, 2:3]
            s_bo = s_pkf[:, 3:4]

            v_full = per.tile([D, EP], BF16)     # resident V^T
            e_full = per.tile([NH, EP], BF16)    # resident exp scores
            zparts = per.tile([NH, NPAIR], F32)  # per-pair Z partials

            # ---------------- pass A ----------------
            psA_ctx = tc.tile_pool(name="psA", bufs=1, space="PSUM")
            psA = psA_ctx.__enter__()
            prev = None      # (s_p, chunk index) for the deferred S8
            ps8 = None

            def do_s8(c):
                nonlocal ps8
                if c % 2 == 0:
                    ps8 = psA.tile([NH, 2 * CH], F32, tag="ps8", bufs=1,
                                   name=f"ps8_{c}")
                nc.tensor.matmul(ps8[:, (c % 2) * CH:(c % 2) * CH + CH],
                                 s_hsum, prev[:], start=True, stop=True)
                if c % 2 == 1:
                    p = c // 2
                    sl2 = slice(p * 2 * CH, (p + 1) * 2 * CH)
                    if p < NPAIR - 1:
                        nc.scalar.activation(e_full[:, sl2], ps8[:], AF.Exp,
                                             bias=0.0, scale=0.25,
                                             accum_out=zparts[:, p:p + 1])
                    else:
                        nc.scalar.activation(e_full[:, sl2], ps8[:], AF.Exp,
                                             bias=0.0, scale=0.25)
                        nc.vector.memset(
                            e_full[:, p * 2 * CH + PTAIL:(p + 1) * 2 * CH], 0.0)
                        nc.vector.tensor_reduce(zparts[:, p:p + 1],
                                                e_full[:, sl2],
                                                axis=mybir.AxisListType.X,
                                                op=ALU.add)

            for c in range(NCH):
                sl = slice(c * CH, (c + 1) * CH)
                if c % (XW // CH) == 0:
                    s_xi = wk.tile([D, XW], BF16, tag="xi")
                    nc.sync.dma_start(s_xi[:], t_xiT[:, c * CH:c * CH + XW])
                    s_xj = wk.tile([D, XW], BF16, tag="xj")
                    nc.sync.dma_start(s_xj[:], t_xjT[:, c * CH:c * CH + XW])
                if c % (EW // CH) == 0:
                    ew = min(EW, EP - c * CH)
                    s_ea = wk.tile([33, EW], BF16, tag="ea")
                    nc.sync.dma_start(s_ea[:, :ew], t_eaT[:, c * CH:c * CH + ew])
                xsl = slice((c % (XW // CH)) * CH, (c % (XW // CH)) * CH + CH)
                esl = slice((c % (EW // CH)) * CH, (c % (EW // CH)) * CH + CH)

                p_q = psA.tile([D, CH], F32, tag="pq", bufs=2)
                nc.tensor.matmul(p_q[:], s_wq, s_xi[:, xsl], start=True, stop=True)
                p_ke = psA.tile([D, CH], F32, tag="pke", bufs=2)
                nc.tensor.matmul(p_ke[:], s_wk, s_xj[:, xsl], start=True, stop=False)
                nc.tensor.matmul(p_ke[:], s_wea, s_ea[:, esl], start=False, stop=True)
                p_v = psA.tile([D, CH], F32, tag="pv", bufs=2)
                nc.tensor.matmul(p_v[:], s_wv, s_xj[:, xsl], start=True, stop=True)
                if c > 0:
                    do_s8(c - 1)

                # V^T chunk -> resident SBUF with bias (ACT)
                nc.scalar.activation(v_full[:, sl], p_v[:], AF.Identity,
                                     bias=s_bv, scale=1.0)
                # KE -> SBUF (DVE copy, bias already folded via ones-row)
                s_ke = mid.tile([D, CH], BF16, tag="ke")
                nc.vector.tensor_copy(s_ke[:], p_ke[:])
                s_p = mid.tile([D, CH], BF16, tag="p")
                nc.vector.scalar_tensor_tensor(s_p[:], p_q[:], s_bq, s_ke[:],
                                               op0=ALU.add, op1=ALU.mult)
                prev = s_p
            do_s8(NCH - 1)

            psA_ctx.__exit__(None, None, None)
            psB_ctx = tc.tile_pool(name="psB", bufs=1, space="PSUM")
            psB = psB_ctx.__enter__()

            # ---------------- global Z (all off the PE critical path) ----
            s_zl = per.tile([NH, 1], F32)
            nc.vector.tensor_reduce(s_zl[:], zparts[:],
                                    axis=mybir.AxisListType.X, op=ALU.add)
            d_zin = dram.tile([NH, 1], F32)
            d_zout = dram.tile([NH, 1], F32)
            nc.sync.dma_start(d_zin[:], s_zl[:])
            nc.gpsimd.collective_compute(
                "AllReduce", ALU.add,
                replica_groups=[list(range(NCORES))],
                ins=[d_zin.opt()],
                outs=[d_zout.opt()],
            )
            # replicate Z[8] -> [128,1] (16x each) with a broadcast DMA read
            s_zexp = per.tile([D, 1], F32)
            nc.sync.dma_start(s_zexp[:], d_zout[:].broadcast_to([NH, DK, 1]))
            s_chd = per.tile([D, 1], F32)
            nc.vector.reciprocal(s_chd[:], s_zexp[:])
            s_wo2 = per.tile([D, D], BF16)
            nc.vector.tensor_scalar(s_wo2[:], s_wo, s_chd[:], None,
                                    op0=ALU.mult)

            # ---------------- pass B ----------------
            us = {}
            def do_pe_u(c):
                sl = slice(c * CH, (c + 1) * CH)
                p_e = psB.tile([D, CH], F32, tag="pe", bufs=2, name=f"pe_{c}")
                nc.tensor.matmul(p_e[:], s_hrep, e_full[:, sl],
                                 start=True, stop=True)
                s_u = mid.tile([D, CH], BF16, tag="u", bufs=PREB + 2,
                               name=f"u_{c}")
                nc.vector.tensor_tensor(s_u[:], p_e[:], v_full[:, sl],
                                        op=ALU.mult)
                us[c] = s_u

            def do_out(c):
                sl = slice(c * CH, (c + 1) * CH)
                p_o = psB.tile([D, CH], F32, tag="pout", bufs=2, name=f"po_{c}")
                nc.tensor.matmul(p_o[:], s_wo2[:], us.pop(c)[:],
                                 start=True, stop=True)
                s_o = mid.tile([D, CH], F32, tag="o")
                nc.scalar.activation(s_o[:], p_o[:], AF.Identity, bias=s_bo,
                                     scale=1.0)
                nc.sync.dma_start(t_out[:, sl], s_o[:])

            # E128/U for the first PREB chunks overlaps the collective
            for c in range(PREB):
                do_pe_u(c)
            for c in range(PREB, NCH):
                do_pe_u(c)
                do_out(c - PREB)
            for c in range(NCH - PREB, NCH):
                do_out(c)
            psB_ctx.__exit__(None, None, None)

    nc.compile()
    _CACHE["nc"] = nc
    return nc


def _pack_constants(wq, bq, wk, bk, wv, bv, we, be, wo, bo):
    Hsum = np.zeros((D, NH), np.float32)
    for hd in range(D):
        Hsum[hd, hd // DK] = 1.0
    pkb = np.zeros((D, 784), np.float32)
    pkb[:, 0:128] = wq
    pkb[:, 128:256] = wk
    pkb[:, 256:384] = wv
    pkb[:, 384:512] = wo
    pkb[:32, 512:640] = we
    pkb[32, 512:640] = bk + be        # bias row (ones-row of eaT hits it)
    pkb[:, 640:648] = Hsum
    pkb[:8, 656:784] = Hsum.T
    pkf = np.zeros((D, 8), np.float32)
    pkf[:, 0] = bq
    pkf[:, 2] = bv
    pkf[:, 3] = bo
    return pkb.astype(BF), pkf


def _run(inputs, trace=False):
    x_i = np.asarray(inputs["x_i"], np.float32)
    x_j = np.asarray(inputs["x_j"], np.float32)
    ea = np.asarray(inputs["edge_attr"], np.float32)
    pkb, pkf = _pack_constants(
        np.asarray(inputs["wq"], np.float32), np.asarray(inputs["bq"], np.float32),
        np.asarray(inputs["wk"], np.float32), np.asarray(inputs["bk"], np.float32),
        np.asarray(inputs["wv"], np.float32), np.asarray(inputs["bv"], np.float32),
        np.asarray(inputs["we"], np.float32), np.asarray(inputs["be"], np.float32),
        np.asarray(inputs["wo"], np.float32), np.asarray(inputs["bo"], np.float32),
    )

    in_maps = []
    for c in range(NCORES):
        sl = slice(c * ES, (c + 1) * ES)
        xiT = np.zeros((D, EP), BF)
        xiT[:, :ES] = x_i[sl].T.astype(BF)
        xjT = np.zeros((D, EP), BF)
        xjT[:, :ES] = x_j[sl].T.astype(BF)
        eaT = np.zeros((33, EP), BF)
        eaT[:32, :ES] = ea[sl].T.astype(BF)
        eaT[32, :ES] = 1.0
        in_maps.append(dict(xiT=xiT, xjT=xjT, eaT=eaT, pkb=pkb, pkf=pkf))

    nc = _build()
    res = run_bass_kernel_spmd(nc, in_maps, list(range(NCORES)), trace=trace)

    out = np.empty((E_FULL, D), np.float32)
    for c in range(NCORES):
        sl = slice(c * ES, (c + 1) * ES)
        out[sl] = res.results[c]["outT"][:, :ES].T
    return out, res.exec_time_ns


def kernel(**inputs) -> np.ndarray:
    return _run(inputs)[0]


# revision 8
# speedup vs baseline: 1.8276x; 1.2836x over previous
"""Trainium2 Bass kernel for nn_MultiHeadAttentionLayer (edge-wise MHA with
global softmax over the edge dimension).

Strategy (8 NeuronCores, data-parallel over edges):
  - Host shards E=250000 edges into 8 shards of 31250, zero-padded to 31744
    (62 chunks x 512), pre-transposed so features land on SBUF partitions,
    and cast to bf16 (PE full-rate dtype + fast weight load).
  - The KE bias (bk+be) is folded into the edge-attr matmul: host appends a
    ones-row to edge_attr^T ([33, E]) and a bias row to we ([33, 128]).
  - Pass A (per 512-edge chunk): QT = wq.T@xiT, KET = wk.T@xjT+weA.T@eaA,
    VT = wv.T@xjT (+bv via ACT, resident SBUF bf16), KE copy to SBUF (DVE),
    P = (QT+bq)*KET (DVE), S8 = Hsum.T@P.  The S8 matmul for chunk c is
    issued in iteration c+1 so the in-order PE never waits on the DVE/ACT
    chain (keeps the PE dense and HAM-warm).  exp(S8/4) runs once per chunk
    PAIR on a [8,1024] PSUM tile -> resident SBUF bf16 + partial Z sums.
  - One AllReduce(add) of Z[8].  Scores are O(1) so no max-subtraction;
    1/Z is folded into wo's rows (wo2, computed off the PE critical path:
    Z -> replicated [128,1] via a broadcast DMA -> DVE reciprocal -> DVE
    scale).  The first PREB chunks of pass B (E128 = Hrep.T@exp, U = E128*V)
    are issued before any wo2-dependent matmul so they overlap the
    collective.
  - Pass B (per chunk): U = E128 * VT (DVE), outT = wo2.T@U + bo -> DRAM
    fp32; the out matmul for chunk c is deferred one iteration like S8.
  - Host gathers and transposes back to [E, 128].
"""
import os
import sys

for _p in ("/opt/trn_rl_repo", "/root/.axon_site/_ro/trn_rl_repo"):
    if os.path.isdir(_p) and _p not in sys.path:
        sys.path.append(_p)

import numpy as np
import ml_dtypes
import concourse.bacc as bacc
import concourse.tile as tile
import concourse.mybir as mybir
from concourse.bass_utils import run_bass_kernel_spmd

F32 = mybir.dt.float32
BF16 = mybir.dt.bfloat16
AF = mybir.ActivationFunctionType
ALU = mybir.AluOpType
BF = ml_dtypes.bfloat16

E_FULL = 250000
NCORES = 8
ES = E_FULL // NCORES          # 31250 edges per core
CH = 512                       # chunk size (PSUM bank width)
NCH = (ES + CH - 1) // CH      # 62 chunks
EP = NCH * CH                  # 31744 padded edges per core
D = 128
NH = 8
DK = 16
XW = 1024                      # xi/xj DMA batch width (2 chunks)
EW = 2048                      # ea DMA batch width (4 chunks)
NPAIR = NCH // 2               # 31 exp pairs
PTAIL = ES - (NPAIR - 1) * 2 * CH   # valid edges in last pair (530)
PREB = 16                      # pass-B chunks prefetched across collective

_CACHE = {}


def _build():
    if "nc" in _CACHE:
        return _CACHE["nc"]

    nc = bacc.Bacc(num_devices=NCORES)

    t_xiT = nc.dram_tensor("xiT", [D, EP], BF16, kind="ExternalInput")
    t_xjT = nc.dram_tensor("xjT", [D, EP], BF16, kind="ExternalInput")
    t_eaT = nc.dram_tensor("eaT", [33, EP], BF16, kind="ExternalInput")
    t_pkb = nc.dram_tensor("pkb", [D, 784], BF16, kind="ExternalInput")
    t_pkf = nc.dram_tensor("pkf", [D, 8], F32, kind="ExternalInput")
    t_out = nc.dram_tensor("outT", [D, EP], F32, kind="ExternalOutput")

    with tile.TileContext(nc) as tc:
        with (
            tc.tile_pool(name="per", bufs=1) as per,      # persistent
            tc.tile_pool(name="wk", bufs=3) as wk,        # streaming loads
            tc.tile_pool(name="mid", bufs=2) as mid,      # intermediates
            tc.tile_pool(name="dram", bufs=1, space="DRAM") as dram,
        ):
            s_pkb = per.tile([D, 784], BF16)
            nc.sync.dma_start(s_pkb[:], t_pkb[:])
            s_wq = s_pkb[:, 0:128]
            s_wk = s_pkb[:, 128:256]
            s_wv = s_pkb[:, 256:384]
            s_wo = s_pkb[:, 384:512]
            s_wea = s_pkb[0:33, 512:640]     # [we ; bk+be]
            s_hsum = s_pkb[:, 640:648]
            s_hrep = s_pkb[0:8, 656:784]

            s_pkf = per.tile([D, 8], F32)
            nc.sync.dma_start(s_pkf[:], t_pkf[:])
            s_bq = s_pkf[:, 0:1]
            s_bv = s_pkf[:, 2:3]
            s_bo = s_pkf[:, 3:4]

            v_full = per.tile([D, EP], BF16)     # resident V^T
            e_full = per.tile([NH, EP], BF16)    # resident exp scores
            zparts = per.tile([NH, NPAIR], F32)  # per-pair Z partials

            # ---------------- pass A ----------------
            psA_ctx = tc.tile_pool(name="psA", bufs=1, space="PSUM")
            psA = psA_ctx.__enter__()
            prev = None      # (s_p, chunk index) for the deferred S8
            ps8 = None

            def do_s8(c):
                nonlocal ps8
                if c % 2 == 0:
                    ps8 = psA.tile([NH, 2 * CH], F32, tag="ps8", bufs=1,
                                   name=f"ps8_{c}")
                nc.tensor.matmul(ps8[:, (c % 2) * CH:(c % 2) * CH + CH],
                                 s_hsum, prev[:], start=True, stop=True)
                if c % 2 == 1:
                    p = c // 2
                    sl2 = slice(p * 2 * CH, (p + 1) * 2 * CH)
                    if p < NPAIR - 1:
                        nc.scalar.activation(e_full[:, sl2], ps8[:], AF.Exp,
                                             bias=0.0, scale=0.25,
                                             accum_out=zparts[:, p:p + 1])
                    else:
                        nc.scalar.activation(e_full[:, sl2], ps8[:], AF.Exp,
                                             bias=0.0, scale=0.25)
                        nc.vector.memset(
                            e_full[:, p * 2 * CH + PTAIL:(p + 1) * 2 * CH], 0.0)
                        nc.vector.tensor_reduce(zparts[:, p:p + 1],
                                                e_full[:, sl2],
                                                axis=mybir.AxisListType.X,
                                                op=ALU.add)

            for c in range(NCH):
                sl = slice(c * CH, (c + 1) * CH)
                if c % (XW // CH) == 0:
                    s_xi = wk.tile([D, XW], BF16, tag="xi")
                    nc.sync.dma_start(s_xi[:], t_xiT[:, c * CH:c * CH + XW])
                    s_xj = wk.tile([D, XW], BF16, tag="xj")
                    nc.sync.dma_start(s_xj[:], t_xjT[:, c * CH:c * CH + XW])
                if c % (EW // CH) == 0:
                    ew = min(EW, EP - c * CH)
                    s_ea = wk.tile([33, EW], BF16, tag="ea")
                    nc.sync.dma_start(s_ea[:, :ew], t_eaT[:, c * CH:c * CH + ew])
                xsl = slice((c % (XW // CH)) * CH, (c % (XW // CH)) * CH + CH)
                esl = slice((c % (EW // CH)) * CH, (c % (EW // CH)) * CH + CH)

                p_q = psA.tile([D, CH], F32, tag="pq", bufs=2)
                nc.tensor.matmul(p_q[:], s_wq, s_xi[:, xsl], start=True, stop=True)
                p_ke = psA.tile([D, CH], F32, tag="pke", bufs=2)
                nc.tensor.matmul(p_ke[:], s_wk, s_xj[:, xsl], start=True, stop=False)
                nc.tensor.matmul(p_ke[:], s_wea, s_ea[:, esl], start=False, stop=True)
                p_v = psA.tile([D, CH], F32, tag="pv", bufs=2)
                nc.tensor.matmul(p_v[:], s_wv, s_xj[:, xsl], start=True, stop=True)
                if c > 0:
                    do_s8(c - 1)

                # V^T chunk -> resident SBUF with bias (ACT)
                nc.scalar.activation(v_full[:, sl], p_v[:], AF.Identity,
                                     bias=s_bv, scale=1.0)
                # KE -> SBUF (DVE copy, bias already folded via ones-row)
                s_ke = mid.tile([D, CH], BF16, tag="ke", bufs=3)
                nc.vector.tensor_copy(s_ke[:], p_ke[:])
                s_p = mid.tile([D, CH], BF16, tag="p", bufs=3)
                nc.vector.scalar_tensor_tensor(s_p[:], p_q[:], s_bq, s_ke[:],
                                               op0=ALU.add, op1=ALU.mult)
                prev = s_p
            do_s8(NCH - 1)

            psA_ctx.__exit__(None, None, None)
            psB_ctx = tc.tile_pool(name="psB", bufs=1, space="PSUM")
            psB = psB_ctx.__enter__()

            # ---------------- global Z (all off the PE critical path) ----
            s_zl = per.tile([NH, 1], F32)
            nc.vector.tensor_reduce(s_zl[:], zparts[:],
                                    axis=mybir.AxisListType.X, op=ALU.add)
            d_zin = dram.tile([NH, 1], F32)
            d_zout = dram.tile([NH, 1], F32)
            nc.sync.dma_start(d_zin[:], s_zl[:])
            nc.gpsimd.collective_compute(
                "AllReduce", ALU.add,
                replica_groups=[list(range(NCORES))],
                ins=[d_zin.opt()],
                outs=[d_zout.opt()],
            )
            # replicate Z[8] -> [128,1] (16x each) with a broadcast DMA read
            s_zexp = per.tile([D, 1], F32)
            nc.sync.dma_start(s_zexp[:], d_zout[:].broadcast_to([NH, DK, 1]))
            s_chd = per.tile([D, 1], F32)
            nc.vector.reciprocal(s_chd[:], s_zexp[:])
            s_wo2 = per.tile([D, D], BF16)
            nc.vector.tensor_scalar(s_wo2[:], s_wo, s_chd[:], None,
                                    op0=ALU.mult)

            # ---------------- pass B ----------------
            us = {}
            def do_pe_u(c):
                sl = slice(c * CH, (c + 1) * CH)
                p_e = psB.tile([D, CH], F32, tag="pe", bufs=4, name=f"pe_{c}")
                nc.tensor.matmul(p_e[:], s_hrep, e_full[:, sl],
                                 start=True, stop=True)
                s_u = mid.tile([D, CH], BF16, tag="u", bufs=PREB + 2,
                               name=f"u_{c}")
                nc.vector.tensor_tensor(s_u[:], p_e[:], v_full[:, sl],
                                        op=ALU.mult)
                us[c] = s_u

            def do_out(c):
                sl = slice(c * CH, (c + 1) * CH)
                p_o = psB.tile([D, CH], F32, tag="pout", bufs=4, name=f"po_{c}")
                nc.tensor.matmul(p_o[:], s_wo2[:], us.pop(c)[:],
                                 start=True, stop=True)
                s_o = mid.tile([D, CH], F32, tag="o", bufs=4)
                nc.scalar.activation(s_o[:], p_o[:], AF.Identity, bias=s_bo,
                                     scale=1.0)
                nc.sync.dma_start(t_out[:, sl], s_o[:])

            # E128/U for the first PREB chunks overlaps the collective
            for c in range(PREB):
                do_pe_u(c)
            for c in range(PREB, NCH):
                do_pe_u(c)
                do_out(c - PREB)
            for c in range(NCH - PREB, NCH):
                do_out(c)
            psB_ctx.__exit__(None, None, None)

    nc.compile()
    _CACHE["nc"] = nc
    return nc


def _pack_constants(wq, bq, wk, bk, wv, bv, we, be, wo, bo):
    Hsum = np.zeros((D, NH), np.float32)
    for hd in range(D):
        Hsum[hd, hd // DK] = 1.0
    pkb = np.zeros((D, 784), np.float32)
    pkb[:, 0:128] = wq
    pkb[:, 128:256] = wk
    pkb[:, 256:384] = wv
    pkb[:, 384:512] = wo
    pkb[:32, 512:640] = we
    pkb[32, 512:640] = bk + be        # bias row (ones-row of eaT hits it)
    pkb[:, 640:648] = Hsum
    pkb[:8, 656:784] = Hsum.T
    pkf = np.zeros((D, 8), np.float32)
    pkf[:, 0] = bq
    pkf[:, 2] = bv
    pkf[:, 3] = bo
    return pkb.astype(BF), pkf


def _run(inputs, trace=False):
    x_i = np.asarray(inputs["x_i"], np.float32)
    x_j = np.asarray(inputs["x_j"], np.float32)
    ea = np.asarray(inputs["edge_attr"], np.float32)
    pkb, pkf = _pack_constants(
        np.asarray(inputs["wq"], np.float32), np.asarray(inputs["bq"], np.float32),
        np.asarray(inputs["wk"], np.float32), np.asarray(inputs["bk"], np.float32),
        np.asarray(inputs["wv"], np.float32), np.asarray(inputs["bv"], np.float32),
        np.asarray(inputs["we"], np.float32), np.asarray(inputs["be"], np.float32),
        np.asarray(inputs["wo"], np.float32), np.asarray(inputs["bo"], np.float32),
    )

    in_maps = []
    for c in range(NCORES):
        sl = slice(c * ES, (c + 1) * ES)
        xiT = np.zeros((D, EP), BF)
        xiT[:, :ES] = x_i[sl].T.astype(BF)
        xjT = np.zeros((D, EP), BF)
        xjT[:, :ES] = x_j[sl].T.astype(BF)
        eaT = np.zeros((33, EP), BF)
        eaT[:32, :ES] = ea[sl].T.astype(BF)
        eaT[32, :ES] = 1.0
        in_maps.append(dict(xiT=xiT, xjT=xjT, eaT=eaT, pkb=pkb, pkf=pkf))

    nc = _build()
    res = run_bass_kernel_spmd(nc, in_maps, list(range(NCORES)), trace=trace)

    out = np.empty((E_FULL, D), np.float32)
    for c in range(NCORES):
        sl = slice(c * ES, (c + 1) * ES)
        out[sl] = res.results[c]["outT"][:, :ES].T
    return out, res.exec_time_ns


def kernel(**inputs) -> np.ndarray:
    return _run(inputs)[0]


# revision 9
# speedup vs baseline: 1.9207x; 1.0509x over previous
"""Trainium2 Bass kernel for nn_MultiHeadAttentionLayer (edge-wise MHA with
global softmax over the edge dimension).

Strategy (8 NeuronCores, data-parallel over edges):
  - Host shards E=250000 edges into 8 shards of 31250, zero-padded to 31744
    (62 chunks x 512), pre-transposed so features land on SBUF partitions,
    and cast to bf16 (PE full-rate dtype + fast weight load).
  - The KE bias (bk+be) is folded into the edge-attr matmul: host appends a
    ones-row to edge_attr^T ([33, E]) and a bias row to we ([33, 128]).
  - Pass A (per 512-edge chunk): QT = wq.T@xiT, KET = wk.T@xjT+weA.T@eaA,
    VT = wv.T@xjT (+bv via ACT, resident SBUF bf16), KE copy to SBUF (DVE),
    P = (QT+bq)*KET (DVE), S8 = Hsum.T@P.  The S8 matmul for chunk c is
    issued in iteration c+1 so the in-order PE never waits on the DVE/ACT
    chain (keeps the PE dense and HAM-warm).  exp(S8/4) runs once per chunk
    PAIR on a [8,1024] PSUM tile -> resident SBUF bf16 + partial Z sums.
  - One AllReduce(add) of Z[8].  Scores are O(1) so no max-subtraction;
    1/Z is folded into wo's rows (wo2, computed off the PE critical path:
    Z -> replicated [128,1] via a broadcast DMA -> DVE reciprocal -> DVE
    scale).  The first PREB chunks of pass B (E128 = Hrep.T@exp, U = E128*V)
    are issued before any wo2-dependent matmul so they overlap the
    collective.
  - Pass B (per chunk): U = E128 * VT (DVE), outT = wo2.T@U + bo -> DRAM
    fp32; the out matmul for chunk c is deferred one iteration like S8.
  - Host gathers and transposes back to [E, 128].
"""
import os
import sys

for _p in ("/opt/trn_rl_repo", "/root/.axon_site/_ro/trn_rl_repo"):
    if os.path.isdir(_p) and _p not in sys.path:
        sys.path.append(_p)

import numpy as np
import ml_dtypes
import concourse.bacc as bacc
import concourse.tile as tile
import concourse.mybir as mybir
from concourse.bass_utils import run_bass_kernel_spmd

F32 = mybir.dt.float32
BF16 = mybir.dt.bfloat16
AF = mybir.ActivationFunctionType
ALU = mybir.AluOpType
BF = ml_dtypes.bfloat16

E_FULL = 250000
NCORES = 8
ES = E_FULL // NCORES          # 31250 edges per core
CH = 512                       # chunk size (PSUM bank width)
NCH = (ES + CH - 1) // CH      # 62 chunks
EP = NCH * CH                  # 31744 padded edges per core
D = 128
NH = 8
DK = 16
XW = 1024                      # xi/xj DMA batch width (2 chunks)
EW = 2048                      # ea DMA batch width (4 chunks)
NPAIR = NCH // 2               # 31 exp pairs
PTAIL = ES - (NPAIR - 1) * 2 * CH   # valid edges in last pair (530)
PREB = 16                      # pass-B chunks prefetched across collective
ARSPLIT = 24                   # Z pairs in the early (hidden) AllReduce

_CACHE = {}


def _build():
    if "nc" in _CACHE:
        return _CACHE["nc"]

    nc = bacc.Bacc(num_devices=NCORES)

    t_xiT = nc.dram_tensor("xiT", [D, EP], BF16, kind="ExternalInput")
    t_xjT = nc.dram_tensor("xjT", [D, EP], BF16, kind="ExternalInput")
    t_eaT = nc.dram_tensor("eaT", [33, EP], BF16, kind="ExternalInput")
    t_pkb = nc.dram_tensor("pkb", [D, 912], BF16, kind="ExternalInput")
    t_pkf = nc.dram_tensor("pkf", [D, 8], F32, kind="ExternalInput")
    t_out = nc.dram_tensor("outT", [D, EP], F32, kind="ExternalOutput")

    with tile.TileContext(nc) as tc:
        with (
            tc.tile_pool(name="per", bufs=1) as per,      # persistent
            tc.tile_pool(name="wk", bufs=3) as wk,        # streaming loads
            tc.tile_pool(name="mid", bufs=2) as mid,      # intermediates
            tc.tile_pool(name="dram", bufs=1, space="DRAM") as dram,
        ):
            s_pkb = per.tile([D, 912], BF16)
            nc.sync.dma_start(s_pkb[:], t_pkb[:])
            s_wq = s_pkb[:, 0:128]
            s_wk = s_pkb[:, 128:256]
            s_wv = s_pkb[:, 256:384]
            s_wo = s_pkb[:, 384:512]
            s_wea = s_pkb[0:33, 512:640]     # [we ; bk+be]
            s_hsum = s_pkb[:, 640:768]
            s_hrep = s_pkb[0:8, 784:912]

            s_pkf = per.tile([D, 8], F32)
            nc.sync.dma_start(s_pkf[:], t_pkf[:])
            s_bq = s_pkf[:, 0:1]
            s_bv = s_pkf[:, 2:3]
            s_bo = s_pkf[:, 3:4]

            v_full = per.tile([D, EP], BF16)     # resident V^T
            e_full = per.tile([NH, EP], BF16)    # resident exp scores
            zparts = per.tile([NH, NPAIR], F32)  # per-pair Z partials

            # ---------------- pass A ----------------
            psA_ctx = tc.tile_pool(name="psA", bufs=1, space="PSUM")
            psA = psA_ctx.__enter__()
            # PE pre-warm: ~4us of dummy matmuls while the first DMAs land,
            # so HAM reaches K=8/8 before the real stream starts.
            warm = per.tile([D, CH], BF16)
            nc.vector.memset(warm[:], 0.0)
            p_warm = psA.tile([D, CH], F32, tag="pq", bufs=2, name="p_warm")
            for _ in range(18):
                nc.tensor.matmul(p_warm[:], warm[:, 0:128], warm[:],
                                 start=True, stop=True)
            prev = None      # (s_p, chunk index) for the deferred S8
            ps8 = None

            def do_s8(c):
                nonlocal ps8
                if c % 2 == 0:
                    ps8 = psA.tile([D, 2 * CH], F32, tag="ps8", bufs=1,
                                   name=f"ps8_{c}")
                nc.tensor.matmul(ps8[:, (c % 2) * CH:(c % 2) * CH + CH],
                                 s_hsum, prev[:], start=True, stop=True)
                if c % 2 == 1:
                    p = c // 2
                    sl2 = slice(p * 2 * CH, (p + 1) * 2 * CH)
                    if p < NPAIR - 1:
                        nc.scalar.activation(e_full[:, sl2], ps8[0:NH, :], AF.Exp,
                                             bias=0.0, scale=0.25,
                                             accum_out=zparts[:, p:p + 1])
                    else:
                        nc.scalar.activation(e_full[:, sl2], ps8[0:NH, :], AF.Exp,
                                             bias=0.0, scale=0.25)
                        nc.vector.memset(
                            e_full[:, p * 2 * CH + PTAIL:(p + 1) * 2 * CH], 0.0)
                        nc.vector.tensor_reduce(zparts[:, p:p + 1],
                                                e_full[:, sl2],
                                                axis=mybir.AxisListType.X,
                                                op=ALU.add)

            for c in range(NCH):
                sl = slice(c * CH, (c + 1) * CH)
                if c % (XW // CH) == 0:
                    s_xi = wk.tile([D, XW], BF16, tag="xi")
                    nc.sync.dma_start(s_xi[:], t_xiT[:, c * CH:c * CH + XW])
                    s_xj = wk.tile([D, XW], BF16, tag="xj")
                    nc.sync.dma_start(s_xj[:], t_xjT[:, c * CH:c * CH + XW])
                if c % (EW // CH) == 0:
                    ew = min(EW, EP - c * CH)
                    s_ea = wk.tile([33, EW], BF16, tag="ea")
                    nc.sync.dma_start(s_ea[:, :ew], t_eaT[:, c * CH:c * CH + ew])
                xsl = slice((c % (XW // CH)) * CH, (c % (XW // CH)) * CH + CH)
                esl = slice((c % (EW // CH)) * CH, (c % (EW // CH)) * CH + CH)

                p_q = psA.tile([D, CH], F32, tag="pq", bufs=2)
                nc.tensor.matmul(p_q[:], s_wq, s_xi[:, xsl], start=True, stop=True)
                p_ke = psA.tile([D, CH], F32, tag="pke", bufs=2)
                nc.tensor.matmul(p_ke[:], s_wk, s_xj[:, xsl], start=True, stop=False)
                nc.tensor.matmul(p_ke[:], s_wea, s_ea[:, esl], start=False, stop=True)
                p_v = psA.tile([D, CH], F32, tag="pv", bufs=2)
                nc.tensor.matmul(p_v[:], s_wv, s_xj[:, xsl], start=True, stop=True)
                if c > 0:
                    do_s8(c - 1)

                # V^T chunk -> resident SBUF with bias (ACT)
                nc.scalar.activation(v_full[:, sl], p_v[:], AF.Identity,
                                     bias=s_bv, scale=1.0)
                # KE -> SBUF (DVE copy, bias already folded via ones-row)
                s_ke = mid.tile([D, CH], BF16, tag="ke", bufs=3)
                nc.vector.tensor_copy(s_ke[:], p_ke[:])
                s_p = mid.tile([D, CH], BF16, tag="p", bufs=3)
                nc.vector.scalar_tensor_tensor(s_p[:], p_q[:], s_bq, s_ke[:],
                                               op0=ALU.add, op1=ALU.mult)
                prev = s_p
                if c == 2 * ARSPLIT + 1:
                    # early AllReduce over Z pairs 0..ARSPLIT-1 (hidden
                    # under the tail of pass A)
                    s_zl1 = per.tile([NH, 1], F32)
                    nc.vector.tensor_reduce(s_zl1[:], zparts[:, :ARSPLIT],
                                            axis=mybir.AxisListType.X,
                                            op=ALU.add)
                    d_zin1 = dram.tile([NH, 1], F32)
                    d_zout1 = dram.tile([NH, 1], F32)
                    nc.sync.dma_start(d_zin1[:], s_zl1[:])
                    nc.gpsimd.collective_compute(
                        "AllReduce", ALU.add,
                        replica_groups=[list(range(NCORES))],
                        ins=[d_zin1.opt()],
                        outs=[d_zout1.opt()],
                    )
            do_s8(NCH - 1)

            psA_ctx.__exit__(None, None, None)
            psB_ctx = tc.tile_pool(name="psB", bufs=1, space="PSUM")
            psB = psB_ctx.__enter__()

            # ---------------- global Z part 2 ----
            s_zl2 = per.tile([NH, 1], F32)
            nc.vector.tensor_reduce(s_zl2[:], zparts[:, ARSPLIT:],
                                    axis=mybir.AxisListType.X, op=ALU.add)
            d_zin2 = dram.tile([NH, 1], F32)
            d_zout2 = dram.tile([NH, 1], F32)
            nc.sync.dma_start(d_zin2[:], s_zl2[:])
            nc.gpsimd.collective_compute(
                "AllReduce", ALU.add,
                replica_groups=[list(range(NCORES))],
                ins=[d_zin2.opt()],
                outs=[d_zout2.opt()],
            )
            # replicate Z[8] -> [128,1] (16x each) with broadcast DMA reads
            s_zexp = per.tile([D, 1], F32)
            nc.sync.dma_start(s_zexp[:], d_zout1[:].broadcast_to([NH, DK, 1]))
            s_zexp2 = per.tile([D, 1], F32)
            nc.sync.dma_start(s_zexp2[:], d_zout2[:].broadcast_to([NH, DK, 1]))
            s_zsum = per.tile([D, 1], F32)
            nc.vector.tensor_tensor(s_zsum[:], s_zexp[:], s_zexp2[:],
                                    op=ALU.add)
            s_chd = per.tile([D, 1], F32)
            nc.vector.reciprocal(s_chd[:], s_zsum[:])
            s_wo2 = per.tile([D, D], BF16)
            nc.vector.tensor_scalar(s_wo2[:], s_wo, s_chd[:], None,
                                    op0=ALU.mult)

            # ---------------- pass B ----------------
            us = {}
            def do_pe_u(c):
                sl = slice(c * CH, (c + 1) * CH)
                p_e = psB.tile([D, CH], F32, tag="pe", bufs=4, name=f"pe_{c}")
                nc.tensor.matmul(p_e[:], s_hrep, e_full[:, sl],
                                 start=True, stop=True)
                s_u = mid.tile([D, CH], BF16, tag="u", bufs=PREB + 2,
                               name=f"u_{c}")
                nc.vector.tensor_tensor(s_u[:], p_e[:], v_full[:, sl],
                                        op=ALU.mult)
                us[c] = s_u

            def do_out(c):
                sl = slice(c * CH, (c + 1) * CH)
                p_o = psB.tile([D, CH], F32, tag="pout", bufs=4, name=f"po_{c}")
                nc.tensor.matmul(p_o[:], s_wo2[:], us.pop(c)[:],
                                 start=True, stop=True)
                s_o = mid.tile([D, CH], F32, tag="o", bufs=4)
                nc.scalar.activation(s_o[:], p_o[:], AF.Identity, bias=s_bo,
                                     scale=1.0)
                nc.sync.dma_start(t_out[:, sl], s_o[:])

            # E128/U for the first PREB chunks overlaps the collective
            for c in range(PREB):
                do_pe_u(c)
            for c in range(PREB, NCH):
                do_pe_u(c)
                do_out(c - PREB)
            for c in range(NCH - PREB, NCH):
                do_out(c)
            psB_ctx.__exit__(None, None, None)

    nc.compile()
    _CACHE["nc"] = nc
    return nc


def _pack_constants(wq, bq, wk, bk, wv, bv, we, be, wo, bo):
    Hsum = np.zeros((D, NH), np.float32)
    for hd in range(D):
        Hsum[hd, hd // DK] = 1.0
    pkb = np.zeros((D, 912), np.float32)
    pkb[:, 0:128] = wq
    pkb[:, 128:256] = wk
    pkb[:, 256:384] = wv
    pkb[:, 384:512] = wo
    pkb[:32, 512:640] = we
    pkb[32, 512:640] = bk + be        # bias row (ones-row of eaT hits it)
    pkb[:, 640:648] = Hsum
    pkb[:8, 784:912] = Hsum.T
    pkf = np.zeros((D, 8), np.float32)
    pkf[:, 0] = bq
    pkf[:, 2] = bv
    pkf[:, 3] = bo
    return pkb.astype(BF), pkf


def _run(inputs, trace=False):
    x_i = np.asarray(inputs["x_i"], np.float32)
    x_j = np.asarray(inputs["x_j"], np.float32)
    ea = np.asarray(inputs["edge_attr"], np.float32)
    pkb, pkf = _pack_constants(
        np.asarray(inputs["wq"], np.float32), np.asarray(inputs["bq"], np.float32),
        np.asarray(inputs["wk"], np.float32), np.asarray(inputs["bk"], np.float32),
        np.asarray(inputs["wv"], np.float32), np.asarray(inputs["bv"], np.float32),
        np.asarray(inputs["we"], np.float32), np.asarray(inputs["be"], np.float32),
        np.asarray(inputs["wo"], np.float32), np.asarray(inputs["bo"], np.float32),
    )

    in_maps = []
    for c in range(NCORES):
        sl = slice(c * ES, (c + 1) * ES)
        xiT = np.zeros((D, EP), BF)
        xiT[:, :ES] = x_i[sl].T.astype(BF)
        xjT = np.zeros((D, EP), BF)
        xjT[:, :ES] = x_j[sl].T.astype(BF)
        eaT = np.zeros((33, EP), BF)
        eaT[:32, :ES] = ea[sl].T.astype(BF)
        eaT[32, :ES] = 1.0
        in_maps.append(dict(xiT=xiT, xjT=xjT, eaT=eaT, pkb=pkb, pkf=pkf))

    nc = _build()
    res = run_bass_kernel_spmd(nc, in_maps, list(range(NCORES)), trace=trace)

    out = np.empty((E_FULL, D), np.float32)
    for c in range(NCORES):
        sl = slice(c * ES, (c + 1) * ES)
        out[sl] = res.results[c]["outT"][:, :ES].T
    return out, res.exec_time_ns


def kernel(**inputs) -> np.ndarray:
    return _run(inputs)[0]
